# revision 1
# baseline (speedup 1.0000x reference)
"""Trainium2 Bass kernel for nn_Block_609885356204 (moe_routing).

Self-contained: accepts FULL inputs, shards across 8 NeuronCores, returns FULL
output.

Sharding strategy (hardcoded):
  - Tokens are assigned to cores STRIDED: token t of batch b -> core t % 8,
    local index i = t // 8.  Each core owns 512 tokens (256 per batch,
    b-major locally).  This balances causal attention perfectly and makes the
    SPMD program identical across cores (per-core differences are data only:
    input shards + causal masks).
  - Attention: sequence-parallel.  Each core computes q/k/v for its own
    tokens; k^T and v are AllGathered; each core computes full multi-head
    causal attention + output projection + LN1 + residual for its 512 tokens.
  - Router: computed locally on own tokens in exact fp32, gates AllGathered.
  - Experts: expert-parallel, dense.  Core e owns expert e (W1[e], W2[e] in
    bf16, resident in SBUF), computes gelu(x1 @ W1e + b1e) @ W2e + b2e for
    ALL 4096 tokens (x1^T AllGathered in bf16), scales by its gate column,
    and the weighted contributions are combined with a ReduceScatter.
  - LN2 + residual computed on the own-token slice; host reassembles.

Matmul dtypes: float32r (11-bit-mantissa, full PE rate at N>=256) for
attention, exact fp32 for the router path, bf16 for the expert FFN.
"""

import numpy as np
import ml_dtypes
from contextlib import ExitStack

import concourse.bacc as bacc
import concourse.bass as bass
import concourse.mybir as mybir
import concourse.tile as tile
from concourse.masks import make_identity
from concourse.bass_utils import run_bass_kernel_spmd

FP32 = mybir.dt.float32
F32R = mybir.dt.float32r
BF16 = mybir.dt.bfloat16
AX = mybir.AxisListType
OP = mybir.AluOpType
ACT = mybir.ActivationFunctionType

B, T, C, H, HD, E, FF = 2, 2048, 1024, 16, 64, 8, 4096
NCORES = 8
TL = (B * T) // NCORES          # 512 local tokens per core
TB = TL // B                    # 256 per batch
P = 128
CF = C // P                     # 8 feature tiles
VA = H * (HD + 2)               # 1056: v + ones col + pad (f32r needs even/8B-aligned dst)
FFH = FF // 2                   # 2048: ff half per weight-residency pass
SCALE = float(C) ** -0.5

_CACHE = {}


def _build(reps=1):
    nc = bacc.Bacc(None, target_bir_lowering=False)

    def inp(name, shape, dt):
        return nc.declare_dram_parameter(name, list(shape), dt, isOutput=False)

    xl = inp("xl", (TL, C), FP32)          # own tokens, token-major
    xlT = inp("xlT", (C, TL), F32R)        # own tokens, feature-major
    wq = inp("wq", (C, C), F32R)           # [C, H*HD]
    wk = inp("wk", (C, C), F32R)
    wv = inp("wv", (C, C), F32R)
    wo = inp("wo", (C, C), F32R)
    bo = inp("bo", (1, C), F32R)
    masks = inp("masks", (NCORES, P, P), FP32)   # per-source-rank diag masks
    wrwn = inp("wrwn", (C, 2 * E), FP32)
    brbn = inp("brbn", (1, 2 * E), FP32)
    noise = inp("noise", (TL, E), FP32)
    esel = inp("esel", (1, E), FP32)             # one-hot expert selector
    ln1g = inp("ln1g", (1, C), FP32)
    ln1b = inp("ln1b", (1, C), FP32)
    ln2g = inp("ln2g", (1, C), FP32)
    ln2b = inp("ln2b", (1, C), FP32)
    w1 = inp("w1", (C, FF), BF16)                # own expert
    b1 = inp("b1", (FF,), FP32)
    w2 = inp("w2", (FF, C), BF16)
    b2 = inp("b2", (1, C), BF16)

    out_sl = nc.declare_dram_parameter("out_slice", [TL, C], FP32, isOutput=True)

    # internal DRAM for collectives
    cc_kT_in = nc.dram_tensor("cc_kT_in", [C, TL], F32R)
    cc_kT_out = nc.dram_tensor("cc_kT_out", [NCORES * C, TL], F32R,
                               addr_space="Shared")
    cc_v_in = nc.dram_tensor("cc_v_in", [TL, VA], F32R)
    cc_v_out = nc.dram_tensor("cc_v_out", [NCORES * TL, VA], F32R,
                              addr_space="Shared")
    cc_g_in = nc.dram_tensor("cc_g_in", [TL, E], FP32)
    cc_g_out = nc.dram_tensor("cc_g_out", [NCORES * TL, E], FP32,
                              addr_space="Shared")
    cc_x_in = nc.dram_tensor("cc_x_in", [C, TL], BF16)
    cc_x_out = nc.dram_tensor("cc_x_out", [NCORES * C, TL], BF16,
                              addr_space="Shared")
    cc_rs_in = nc.dram_tensor("cc_rs_in", [NCORES * TL, C], BF16)
    cc_rs_out = nc.dram_tensor("cc_rs_out", [TL, C], BF16)
    RG = [list(range(NCORES))]

    def bcast(handle, rows, cols):
        a = handle.ap()
        return bass.AP(tensor=a.tensor, offset=a.offset,
                       ap=[[0, rows]] + list(a.ap)[1:])

    with tile.TileContext(nc) as tc, ExitStack() as octx:
        const = octx.enter_context(tc.tile_pool(name="const", bufs=1))
        ident = const.tile([P, P], FP32)
        make_identity(nc, ident)
        ones_f = const.tile([1, P], FP32)
        nc.vector.memset(ones_f, 1.0)
        ones_r = const.tile([1, P], F32R)
        nc.vector.tensor_copy(out=ones_r[:], in_=ones_f[:])
        ones2 = const.tile([P, H, 2], FP32)
        nc.vector.memset(ones2, 1.0)
        ones_h = const.tile([1, P], BF16)
        nc.vector.memset(ones_h, 1.0)
        eps_t = const.tile([P, 1], FP32)
        nc.vector.memset(eps_t, 1e-5)
        g1b = const.tile([P, C], FP32)
        nc.gpsimd.dma_start(out=g1b, in_=bcast(ln1g, P, C))
        b1b = const.tile([P, C], FP32)
        nc.gpsimd.dma_start(out=b1b, in_=bcast(ln1b, P, C))
        g2b = const.tile([P, C], FP32)
        nc.gpsimd.dma_start(out=g2b, in_=bcast(ln2g, P, C))
        b2b = const.tile([P, C], FP32)
        nc.gpsimd.dma_start(out=b2b, in_=bcast(ln2b, P, C))
        brbn_b = const.tile([P, 2 * E], FP32)
        nc.gpsimd.dma_start(out=brbn_b, in_=bcast(brbn, P, 2 * E))
        esel_b = const.tile([P, E], FP32)
        nc.gpsimd.dma_start(out=esel_b, in_=bcast(esel, P, E))
        mask_t = const.tile([P, NCORES, P], FP32)
        nc.sync.dma_start(out=mask_t,
                          in_=masks.ap().rearrange("r p q -> p r q"))
        bo_sb = const.tile([1, C], F32R)
        nc.sync.dma_start(out=bo_sb, in_=bo.ap())
        b2_sb = const.tile([1, C], BF16)
        nc.sync.dma_start(out=b2_sb, in_=b2.ap())
        wrwn_sb = const.tile([P, CF, 2 * E], FP32)
        nc.sync.dma_start(out=wrwn_sb,
                          in_=wrwn.ap().rearrange("(f p) e -> p f e", p=P))
        b1_sb = const.tile([P, FF // P], FP32)
        nc.sync.dma_start(out=b1_sb,
                          in_=b1.ap().rearrange("(o p) -> p o", p=P))

        persist = octx.enter_context(tc.tile_pool(name="persist", bufs=1))
        qT = [persist.tile([P, TL], F32R, name=f"qT{m}") for m in range(CF)]
        x1sb = [persist.tile([P, C], FP32, name=f"x1_{m}") for m in range(4)]
        attn_sb = [persist.tile([P, C], FP32, name=f"attn{m}") for m in range(4)]

        for _rep in range(reps):
         # ---------------- Phase A: q/k/v projections ----------------
         with ExitStack() as ctx:
             pa = ctx.enter_context(tc.tile_pool(name="pa", bufs=1))
             ev = ctx.enter_context(tc.tile_pool(name="ev", bufs=4))
             pps = ctx.enter_context(tc.tile_pool(name="pps", bufs=4, space="PSUM"))
             xt = []
             for f in range(CF):
                 t = pa.tile([P, TL], F32R, name=f"xt{f}")
                 nc.sync.dma_start(out=t, in_=xlT.ap()[f * P:(f + 1) * P, :])
                 xt.append(t)
             with ExitStack() as c2:
                 pw = c2.enter_context(tc.tile_pool(name="pwv", bufs=1))
                 wv_t = []
                 for f in range(CF):
                     t = pw.tile([P, C], F32R, name=f"wv{f}")
                     nc.sync.dma_start(out=t, in_=wv.ap()[f * P:(f + 1) * P, :])
                     wv_t.append(t)
                 for mt in range(4):
                     va = ev.tile([P, H, HD + 2], F32R, name="vaug")
                     nc.vector.tensor_copy(out=va[:, :, HD:HD + 2], in_=ones2[:])
                     for n in range(2):
                         ps_ = pps.tile([P, 512], FP32, name="v_ps")
                         for f in range(CF):
                             nc.tensor.matmul(
                                 ps_[:], xt[f][:, mt * P:(mt + 1) * P],
                                 wv_t[f][:, n * 512:(n + 1) * 512],
                                 start=(f == 0), stop=(f == CF - 1))
                         nc.vector.tensor_copy(
                             out=va[:, n * 8:(n + 1) * 8, 0:HD],
                             in_=ps_[:].rearrange("p (h d) -> p h d", d=HD))
                     nc.sync.dma_start(out=cc_v_in.ap()[mt * P:(mt + 1) * P, :],
                                       in_=va[:].rearrange("p h d -> p (h d)"))
             nc.gpsimd.collective_compute(
                 "AllGather", OP.bypass, replica_groups=RG,
                 ins=[cc_v_in.ap()], outs=[cc_v_out.ap()])
             with ExitStack() as c2:
                 pw = c2.enter_context(tc.tile_pool(name="pwk", bufs=1))
                 wk_t = []
                 for f in range(CF):
                     t = pw.tile([P, C], F32R, name=f"wk{f}")
                     nc.sync.dma_start(out=t, in_=wk.ap()[f * P:(f + 1) * P, :])
                     wk_t.append(t)
                 for m in range(CF):
                     ps2 = pps.tile([P, TL], FP32, name="qk_ps")
                     for f in range(CF):
                         nc.tensor.matmul(ps2[:], wk_t[f][:, m * P:(m + 1) * P],
                                          xt[f][:], start=(f == 0),
                                          stop=(f == CF - 1))
                     kt_ev = ev.tile([P, TL], F32R, name="kt_ev")
                     nc.vector.tensor_copy(out=kt_ev[:], in_=ps2[:])
                     nc.sync.dma_start(out=cc_kT_in.ap()[m * P:(m + 1) * P, :],
                                       in_=kt_ev[:])
             nc.gpsimd.collective_compute(
                 "AllGather", OP.bypass, replica_groups=RG,
                 ins=[cc_kT_in.ap()], outs=[cc_kT_out.ap()])
             with ExitStack() as c2:
                 pw = c2.enter_context(tc.tile_pool(name="pwq", bufs=1))
                 wq_t = []
                 for f in range(CF):
                     t = pw.tile([P, C], F32R, name=f"wq{f}")
                     nc.sync.dma_start(out=t, in_=wq.ap()[f * P:(f + 1) * P, :])
                     wq_t.append(t)
                 for m in range(CF):
                     ps_ = pps.tile([P, TL], FP32, name="qk_ps")
                     for f in range(CF):
                         nc.tensor.matmul(ps_[:], wq_t[f][:, m * P:(m + 1) * P],
                                          xt[f][:], start=(f == 0),
                                          stop=(f == CF - 1))
                     nc.vector.tensor_copy(out=qT[m][:], in_=ps_[:])

         # ---------------- Phase B1: causal attention ----------------
         with ExitStack() as ctx:
             ktp = ctx.enter_context(tc.tile_pool(name="ktp", bufs=10))
             vtp = ctx.enter_context(tc.tile_pool(name="vtp", bufs=24))
             ex0p = ctx.enter_context(tc.tile_pool(name="ex0p", bufs=12))
             ex1p = ctx.enter_context(tc.tile_pool(name="ex1p", bufs=12))
             tmpp = ctx.enter_context(tc.tile_pool(name="tmpp", bufs=6))
             sps = ctx.enter_context(tc.tile_pool(name="sps", bufs=2, space="PSUM"))
             s1s = ctx.enter_context(tc.tile_pool(name="s1s", bufs=2, space="PSUM"))
             avs = ctx.enter_context(tc.tile_pool(name="avs", bufs=3, space="PSUM"))
             for b in range(B):
                 for hp in range(H // 2):
                     kts, vts = {}, {}
                     for r in range(NCORES):
                         kt = ktp.tile([P, TB], F32R, name="kt")
                         nc.sync.dma_start(
                             out=kt,
                             in_=cc_kT_out.ap()[r * C + hp * P:r * C + (hp + 1) * P,
                                                b * TB:(b + 1) * TB])
                         kts[r] = kt
                         for ik in range(2):
                             vt = vtp.tile([P, 132], F32R, name="vt")
                             row0 = r * TL + b * TB + ik * P
                             nc.sync.dma_start(
                                 out=vt,
                                 in_=cc_v_out.ap()[row0:row0 + P,
                                                   hp * 132:(hp + 1) * 132])
                             vts[(r, ik)] = vt
                     av0 = avs.tile([P, 132], FP32, name="av")
                     av1 = avs.tile([P, 132], FP32, name="av")
                     for hh in range(2):
                         bp = hh * HD
                         exps0, exps1 = {}, {}
                         for r in range(NCORES):
                             sp = sps.tile([P, TB], FP32, name="sc")
                             nc.tensor.matmul(
                                 sp[:], kts[r][bp:bp + HD, 0:P],
                                 qT[hp][bp:bp + HD, b * TB:(b + 1) * TB],
                                 start=True, stop=True)
                             e0 = ex0p.tile([P, TB], F32R, name="e0")
                             nc.scalar.activation(e0[:, P:TB], sp[:, P:TB],
                                                  ACT.Exp, scale=SCALE)
                             tmp = tmpp.tile([P, P], FP32, name="tmp")
                             nc.scalar.activation(tmp[:], sp[:, 0:P],
                                                  ACT.Exp, scale=SCALE)
                             nc.vector.tensor_mul(e0[:, 0:P], tmp[:],
                                                  mask_t[:, r, :])
                             exps0[r] = e0
                             s1 = s1s.tile([P, P], FP32, name="s1")
                             nc.tensor.matmul(
                                 s1[:], kts[r][bp:bp + HD, P:TB],
                                 qT[hp][bp:bp + HD, b * TB + P:(b + 1) * TB],
                                 start=True, stop=True)
                             tmp2 = tmpp.tile([P, P], FP32, name="tmp")
                             nc.scalar.activation(tmp2[:], s1[:],
                                                  ACT.Exp, scale=SCALE)
                             e1 = ex1p.tile([P, P], F32R, name="e1")
                             nc.vector.tensor_mul(e1[:], tmp2[:], mask_t[:, r, :])
                             exps1[r] = e1
                         co = hh * (HD + 2)
                         for r in range(NCORES):
                             nc.tensor.matmul(
                                 av0[:, co:co + HD + 2], exps0[r][:, 0:P],
                                 vts[(r, 0)][:, co:co + HD + 2],
                                 start=(r == 0), stop=(r == NCORES - 1))
                         for r in range(NCORES):
                             nc.tensor.matmul(
                                 av1[:, co:co + HD + 2], exps0[r][:, P:TB],
                                 vts[(r, 0)][:, co:co + HD + 2],
                                 start=(r == 0), stop=False)
                             nc.tensor.matmul(
                                 av1[:, co:co + HD + 2], exps1[r][:],
                                 vts[(r, 1)][:, co:co + HD + 2],
                                 start=False, stop=(r == NCORES - 1))
                     for q, avp in enumerate((av0, av1)):
                         mtt = b * 2 + q
                         rc = tmpp.tile([P, 2], FP32, name="rc")
                         den = avp[:].rearrange("p (h s) -> p h s", s=HD + 2)[:, :, HD]
                         nc.vector.reciprocal(rc[:], den)
                         for hh in range(2):
                             h = 2 * hp + hh
                             nc.vector.tensor_scalar_mul(
                                 attn_sb[mtt][:, h * HD:(h + 1) * HD],
                                 avp[:, hh * (HD + 2):hh * (HD + 2) + HD],
                                 rc[:, hh:hh + 1])

         # ---------------- Phase B2: proj + LN1 + router ----------------
         with ExitStack() as ctx:
             pw = ctx.enter_context(tc.tile_pool(name="pb2", bufs=1))
             evp = ctx.enter_context(tc.tile_pool(name="evb2", bufs=4))
             tps = ctx.enter_context(tc.tile_pool(name="tps", bufs=3, space="PSUM"))
             prs = ctx.enter_context(tc.tile_pool(name="prs", bufs=2, space="PSUM"))
             wo_t = []
             for f in range(CF):
                 t = pw.tile([P, C], F32R, name=f"wo{f}")
                 nc.sync.dma_start(out=t, in_=wo.ap()[f * P:(f + 1) * P, :])
                 wo_t.append(t)
             attnT = [pw.tile([P, TL], F32R, name=f"attnT{f}") for f in range(CF)]
             for mt in range(4):
                 for f in range(CF):
                     tp = tps.tile([P, P], FP32, name="tp")
                     nc.tensor.transpose(tp[:], attn_sb[mt][:, f * P:(f + 1) * P],
                                         ident[:])
                     nc.vector.tensor_copy(out=attnT[f][:, mt * P:(mt + 1) * P],
                                           in_=tp[:])
             x1T_f = [pw.tile([P, TL], FP32, name=f"x1Tf{f}") for f in range(CF)]
             x1T_h = [pw.tile([P, TL], BF16, name=f"x1Th{f}") for f in range(CF)]
             for mt in range(4):
                 p_sb = evp.tile([P, C], FP32, name="p_sb")
                 for n in range(2):
                     pp = prs.tile([P, 512], FP32, name="pp")
                     nc.tensor.matmul(pp[:], ones_r[:],
                                      bo_sb[:, n * 512:(n + 1) * 512],
                                      start=True, stop=False)
                     for f in range(CF):
                         nc.tensor.matmul(pp[:], attnT[f][:, mt * P:(mt + 1) * P],
                                          wo_t[f][:, n * 512:(n + 1) * 512],
                                          start=False, stop=(f == CF - 1))
                     nc.scalar.copy(out=p_sb[:, n * 512:(n + 1) * 512], in_=pp[:])
                 # LN1 + residual
                 stats = evp.tile([P, 2, 6], FP32, name="stats")
                 for sg in range(2):
                     nc.vector.bn_stats(out=stats[:, sg, :],
                                        in_=p_sb[:, sg * 512:(sg + 1) * 512])
                 mv = evp.tile([P, 2], FP32, name="mv")
                 nc.vector.bn_aggr(out=mv[:], in_=stats[:])
                 std = evp.tile([P, 1], FP32, name="std")
                 nc.scalar.activation(out=std[:], in_=mv[:, 1:2], func=ACT.Sqrt,
                                      bias=eps_t[:])
                 rstd = evp.tile([P, 1], FP32, name="rstd")
                 nc.vector.reciprocal(rstd[:], std[:])
                 xt_l = evp.tile([P, C], FP32, name="xt_l")
                 nc.sync.dma_start(out=xt_l, in_=xl.ap()[mt * P:(mt + 1) * P, :])
                 t1 = evp.tile([P, C], FP32, name="t1")
                 nc.vector.tensor_scalar(out=t1[:], in0=p_sb[:],
                                         scalar1=mv[:, 0:1], scalar2=rstd[:],
                                         op0=OP.subtract, op1=OP.mult)
                 nc.vector.tensor_mul(t1[:], t1[:], g1b[:])
                 nc.vector.tensor_add(t1[:], t1[:], b1b[:])
                 nc.vector.tensor_add(x1sb[mt][:], t1[:], xt_l[:])
                 # x1 transposes: fp32 (router) + bf16 (expert AG)
                 for f in range(CF):
                     tp = tps.tile([P, P], FP32, name="tp")
                     nc.tensor.transpose(tp[:], x1sb[mt][:, f * P:(f + 1) * P],
                                         ident[:])
                     nc.vector.tensor_copy(out=x1T_f[f][:, mt * P:(mt + 1) * P],
                                           in_=tp[:])
                     nc.scalar.copy(out=x1T_h[f][:, mt * P:(mt + 1) * P],
                                    in_=tp[:])
             for f in range(CF):
                 nc.sync.dma_start(out=cc_x_in.ap()[f * P:(f + 1) * P, :],
                                   in_=x1T_h[f][:])
             # router (exact fp32)
             for mt in range(4):
                 rp = prs.tile([P, 2 * E], FP32, name="rp")
                 for f in range(CF):
                     nc.tensor.matmul(rp[:], x1T_f[f][:, mt * P:(mt + 1) * P],
                                      wrwn_sb[:, f, :], start=(f == 0),
                                      stop=(f == CF - 1))
                 lg = evp.tile([P, 2 * E], FP32, name="lg")
                 nc.vector.tensor_add(lg[:], rp[:], brbn_b[:])
                 e_ = evp.tile([P, E], FP32, name="e_")
                 nc.scalar.activation(out=e_[:], in_=lg[:, E:2 * E],
                                      func=ACT.Exp)
                 sp_ = evp.tile([P, E], FP32, name="sp_")
                 nc.scalar.activation(out=sp_[:], in_=e_[:], func=ACT.Ln,
                                      bias=1.0)
                 nz = evp.tile([P, E], FP32, name="nz")
                 nc.sync.dma_start(out=nz, in_=noise.ap()[mt * P:(mt + 1) * P, :])
                 noisy = evp.tile([P, E], FP32, name="noisy")
                 nc.vector.tensor_mul(noisy[:], nz[:], sp_[:])
                 nc.vector.tensor_add(noisy[:], noisy[:], lg[:, 0:E])
                 m1 = evp.tile([P, 1], FP32, name="m1")
                 nc.vector.reduce_max(m1[:], noisy[:], axis=AX.X)
                 nm1 = evp.tile([P, 1], FP32, name="nm1")
                 nc.vector.tensor_scalar_mul(nm1[:], m1[:], -1.0)
                 eq = evp.tile([P, E], FP32, name="eq")
                 nc.vector.tensor_scalar(out=eq[:], in0=noisy[:], scalar1=m1[:],
                                         scalar2=None, op0=OP.is_equal)
                 t2 = evp.tile([P, E], FP32, name="t2")
                 nc.vector.tensor_scalar_mul(t2[:], eq[:], -1e30)
                 nc.vector.tensor_add(t2[:], t2[:], noisy[:])
                 m2 = evp.tile([P, 1], FP32, name="m2")
                 nc.vector.reduce_max(m2[:], t2[:], axis=AX.X)
                 d = evp.tile([P, E], FP32, name="d")
                 nc.scalar.activation(out=d[:], in_=noisy[:], func=ACT.Exp,
                                      bias=nm1[:])
                 em2 = evp.tile([P, 1], FP32, name="em2")
                 nc.scalar.activation(out=em2[:], in_=m2[:], func=ACT.Exp,
                                      bias=nm1[:])
                 den_ = evp.tile([P, 1], FP32, name="den_")
                 nc.scalar.add(out=den_[:], in_=em2[:], add=1.0)
                 rden = evp.tile([P, 1], FP32, name="rden")
                 nc.vector.reciprocal(rden[:], den_[:])
                 ge_ = evp.tile([P, E], FP32, name="ge_")
                 nc.vector.tensor_scalar(out=ge_[:], in0=noisy[:], scalar1=m2[:],
                                         scalar2=None, op0=OP.is_ge)
                 gt = evp.tile([P, E], FP32, name="gt")
                 nc.vector.tensor_mul(gt[:], d[:], ge_[:])
                 nc.vector.tensor_scalar_mul(gt[:], gt[:], rden[:])
                 nc.sync.dma_start(out=cc_g_in.ap()[mt * P:(mt + 1) * P, :],
                                   in_=gt[:])
             nc.gpsimd.collective_compute(
                 "AllGather", OP.bypass, replica_groups=RG,
                 ins=[cc_g_in.ap()], outs=[cc_g_out.ap()])
             nc.gpsimd.collective_compute(
                 "AllGather", OP.bypass, replica_groups=RG,
                 ins=[cc_x_in.ap()], outs=[cc_x_out.ap()])

         # ---------------- Phase C: expert FFN (dense) + RS + LN2 ----------------
         with ExitStack() as ctx:
             xcp = ctx.enter_context(tc.tile_pool(name="xcp", bufs=12))
             htp = ctx.enter_context(tc.tile_pool(name="htp", bufs=20))
             gp = ctx.enter_context(tc.tile_pool(name="gp", bufs=6))
             ctp = ctx.enter_context(tc.tile_pool(name="ctp", bufs=4))
             hs = ctx.enter_context(tc.tile_pool(name="hs", bufs=2, space="PSUM"))
             es = ctx.enter_context(tc.tile_pool(name="es", bufs=4, space="PSUM"))
             for psi in range(2):
                 with ExitStack() as c2:
                     pw = c2.enter_context(tc.tile_pool(name=f"pwf{psi}", bufs=1))
                     w1h = []
                     for f in range(CF):
                         t = pw.tile([P, FFH], BF16, name=f"w1h{f}")
                         nc.sync.dma_start(
                             out=t, in_=w1.ap()[f * P:(f + 1) * P,
                                                psi * FFH:(psi + 1) * FFH])
                         w1h.append(t)
                     w2h = []
                     for k in range(FFH // P):
                         t = pw.tile([P, C], BF16, name=f"w2h{k}")
                         r0 = psi * FFH + k * P
                         nc.sync.dma_start(out=t, in_=w2.ap()[r0:r0 + P, :])
                         w2h.append(t)
                     for cch in range(NCORES):
                         x1c = []
                         for f in range(CF):
                             t = xcp.tile([P, TL], BF16, name="x1c")
                             nc.sync.dma_start(
                                 out=t,
                                 in_=cc_x_out.ap()[cch * C + f * P:
                                                   cch * C + (f + 1) * P, :])
                             x1c.append(t)
                         hts = []
                         for k in range(FFH // P):
                             hps = hs.tile([P, TL], FP32, name="hps")
                             for f in range(CF):
                                 nc.tensor.matmul(hps[:],
                                                  w1h[f][:, k * P:(k + 1) * P],
                                                  x1c[f][:], start=(f == 0),
                                                  stop=(f == CF - 1))
                             ht = htp.tile([P, TL], BF16, name="ht")
                             kk = psi * (FFH // P) + k
                             nc.scalar.activation(out=ht[:], in_=hps[:],
                                                  func=ACT.Gelu,
                                                  bias=b1_sb[:, kk:kk + 1])
                             hts.append(ht)
                         for m in range(4):
                             gt_ = gp.tile([P, E], FP32, name="gt_")
                             nc.sync.dma_start(
                                 out=gt_,
                                 in_=cc_g_out.ap()[cch * TL + m * P:
                                                   cch * TL + (m + 1) * P, :])
                             tg = gp.tile([P, E], FP32, name="tg")
                             nc.vector.tensor_mul(tg[:], gt_[:], esel_b[:])
                             gcol = gp.tile([P, 1], FP32, name="gcol")
                             nc.vector.reduce_sum(gcol[:], tg[:], axis=AX.X)
                             # both n-halves accumulate together so each
                             # hts[k] lhsT is reused by two consecutive MMs
                             eops = [es.tile([P, 512], FP32, name="eop")
                                     for _ in range(2)]
                             for n in range(2):
                                 nc.tensor.matmul(eops[n][:], ones_h[:],
                                                  b2_sb[:, n * 512:(n + 1) * 512],
                                                  start=True, stop=False)
                             for k in range(FFH // P):
                                 for n in range(2):
                                     nc.tensor.matmul(
                                         eops[n][:], hts[k][:, m * P:(m + 1) * P],
                                         w2h[k][:, n * 512:(n + 1) * 512],
                                         start=False, stop=(k == FFH // P - 1))
                             for n in range(2):
                                 cb = ctp.tile([P, 512], BF16, name="cb")
                                 nc.vector.tensor_scalar_mul(cb[:], eops[n][:],
                                                             gcol[:])
                                 dst = cc_rs_in.ap()[cch * TL + m * P:
                                                     cch * TL + (m + 1) * P,
                                                     n * 512:(n + 1) * 512]
                                 if psi == 0:
                                     nc.sync.dma_start(out=dst, in_=cb[:])
                                 else:
                                     nc.gpsimd.dma_start(out=dst, in_=cb[:],
                                                         accum_op=OP.add)
             nc.gpsimd.collective_compute(
                 "ReduceScatter", OP.add, replica_groups=RG,
                 ins=[cc_rs_in.ap()], outs=[cc_rs_out.ap()])
             # LN2 + residual + output
             mp = ctx.enter_context(tc.tile_pool(name="mp", bufs=4))
             for mt in range(4):
                 mo_h = mp.tile([P, C], BF16, name="mo_h")
                 nc.sync.dma_start(out=mo_h,
                                   in_=cc_rs_out.ap()[mt * P:(mt + 1) * P, :])
                 mo = mp.tile([P, C], FP32, name="mo")
                 nc.vector.tensor_copy(out=mo[:], in_=mo_h[:])
                 stats = mp.tile([P, 2, 6], FP32, name="stats2")
                 for sg in range(2):
                     nc.vector.bn_stats(out=stats[:, sg, :],
                                        in_=mo[:, sg * 512:(sg + 1) * 512])
                 mv = mp.tile([P, 2], FP32, name="mv2")
                 nc.vector.bn_aggr(out=mv[:], in_=stats[:])
                 std = mp.tile([P, 1], FP32, name="std2")
                 nc.scalar.activation(out=std[:], in_=mv[:, 1:2], func=ACT.Sqrt,
                                      bias=eps_t[:])
                 rstd = mp.tile([P, 1], FP32, name="rstd2")
                 nc.vector.reciprocal(rstd[:], std[:])
                 t1 = mp.tile([P, C], FP32, name="t1o")
                 nc.vector.tensor_scalar(out=t1[:], in0=mo[:],
                                         scalar1=mv[:, 0:1], scalar2=rstd[:],
                                         op0=OP.subtract, op1=OP.mult)
                 nc.vector.tensor_mul(t1[:], t1[:], g2b[:])
                 nc.vector.tensor_add(t1[:], t1[:], b2b[:])
                 nc.vector.tensor_add(t1[:], t1[:], x1sb[mt][:])
                 nc.sync.dma_start(out=out_sl.ap()[mt * P:(mt + 1) * P, :],
                                   in_=t1[:])

    nc.compile()
    return nc


def _make_in_maps(x, Wq, Wk, Wv, Wo, bo, ln1_g, ln1_b, Wr, br, Wn, bn,
                  W1, b1, W2, b2, ln2_g, ln2_b, noise):
    x = np.asarray(x, np.float32)
    noise = np.asarray(noise, np.float32)
    wq_f = np.ascontiguousarray(
        np.transpose(np.asarray(Wq, np.float32), (1, 0, 2)).reshape(C, C))
    wk_f = np.ascontiguousarray(
        np.transpose(np.asarray(Wk, np.float32), (1, 0, 2)).reshape(C, C))
    wv_f = np.ascontiguousarray(
        np.transpose(np.asarray(Wv, np.float32), (1, 0, 2)).reshape(C, C))
    wo_f = np.ascontiguousarray(np.asarray(Wo, np.float32))
    wrwn_f = np.ascontiguousarray(
        np.concatenate([np.asarray(Wr, np.float32),
                        np.asarray(Wn, np.float32)], axis=1))
    brbn_f = np.concatenate([np.asarray(br, np.float32),
                             np.asarray(bn, np.float32)]).reshape(1, 2 * E)
    W1 = np.asarray(W1, np.float32)
    W2 = np.asarray(W2, np.float32)
    b1 = np.asarray(b1, np.float32)
    b2 = np.asarray(b2, np.float32)

    in_maps = []
    for c in range(NCORES):
        xlc = np.ascontiguousarray(
            np.concatenate([x[0, c::NCORES, :], x[1, c::NCORES, :]], axis=0))
        nzc = np.ascontiguousarray(
            np.concatenate([noise[0, c::NCORES, :], noise[1, c::NCORES, :]],
                           axis=0))
        mk = np.empty((NCORES, P, P), np.float32)
        for r in range(NCORES):
            mk[r] = np.triu(np.ones((P, P), np.float32),
                            0 if r <= c else 1)
        es_ = np.zeros((1, E), np.float32)
        es_[0, c] = 1.0
        in_maps.append({
            "xl": xlc,
            "xlT": np.ascontiguousarray(xlc.T),
            "wq": wq_f, "wk": wk_f, "wv": wv_f, "wo": wo_f,
            "bo": np.asarray(bo, np.float32).reshape(1, C),
            "masks": mk,
            "wrwn": wrwn_f, "brbn": brbn_f,
            "noise": nzc, "esel": es_,
            "ln1g": np.asarray(ln1_g, np.float32).reshape(1, C),
            "ln1b": np.asarray(ln1_b, np.float32).reshape(1, C),
            "ln2g": np.asarray(ln2_g, np.float32).reshape(1, C),
            "ln2b": np.asarray(ln2_b, np.float32).reshape(1, C),
            "w1": W1[c].astype(ml_dtypes.bfloat16),
            "b1": b1[c],
            "w2": W2[c].astype(ml_dtypes.bfloat16),
            "b2": b2[c].reshape(1, C).astype(ml_dtypes.bfloat16),
        })

    return in_maps


def kernel(**inputs):
    if "nc" not in _CACHE:
        _CACHE["nc"] = _build()
    nc = _CACHE["nc"]
    in_maps = _make_in_maps(**inputs)
    res = run_bass_kernel_spmd(nc, in_maps, list(range(NCORES)))
    out = np.empty((B, T, C), np.float32)
    for c in range(NCORES):
        sl = np.asarray(res.results[c]["out_slice"], np.float32)
        out[0, c::NCORES, :] = sl[:TB]
        out[1, c::NCORES, :] = sl[TB:]
    return out



# revision 2
# speedup vs baseline: 1.0821x; 1.0821x over previous
"""Trainium2 Bass kernel for nn_Block_609885356204 (moe_routing).

Self-contained: accepts FULL inputs, shards across 8 NeuronCores, returns FULL
output.

Sharding strategy (hardcoded):
  - Tokens are assigned to cores STRIDED: token t of batch b -> core t % 8,
    local index i = t // 8.  Each core owns 512 tokens (256 per batch,
    b-major locally).  This balances causal attention perfectly and makes the
    SPMD program identical across cores (per-core differences are data only:
    input shards + causal masks).
  - Attention: sequence-parallel.  Each core computes q/k/v for its own
    tokens; k^T and v are AllGathered; each core computes full multi-head
    causal attention + output projection + LN1 + residual for its 512 tokens.
  - Router: computed locally on own tokens in exact fp32, gates AllGathered.
  - Experts: expert-parallel, dense.  Core e owns expert e (W1[e], W2[e] in
    bf16, resident in SBUF), computes gelu(x1 @ W1e + b1e) @ W2e + b2e for
    ALL 4096 tokens (x1^T AllGathered in bf16), scales by its gate column,
    and the weighted contributions are combined with a ReduceScatter.
  - LN2 + residual computed on the own-token slice; host reassembles.

Matmul dtypes: float32r (11-bit-mantissa, full PE rate at N>=256) for
attention, exact fp32 for the router path, bf16 for the expert FFN.
"""

import numpy as np
import ml_dtypes
from contextlib import ExitStack

import concourse.bacc as bacc
import concourse.bass as bass
import concourse.mybir as mybir
import concourse.tile as tile
from concourse.masks import make_identity
from concourse.bass_utils import run_bass_kernel_spmd

FP32 = mybir.dt.float32
F32R = mybir.dt.float32r
FP16 = mybir.dt.float16
BF16 = mybir.dt.bfloat16
AX = mybir.AxisListType
OP = mybir.AluOpType
ACT = mybir.ActivationFunctionType

B, T, C, H, HD, E, FF = 2, 2048, 1024, 16, 64, 8, 4096
NCORES = 8
TL = (B * T) // NCORES          # 512 local tokens per core
TB = TL // B                    # 256 per batch
P = 128
CF = C // P                     # 8 feature tiles
VA = H * (HD + 2)               # 1056: v + ones col + pad (f32r needs even/8B-aligned dst)
FFH = FF // 2                   # 2048: ff half per weight-residency pass
SCALE = float(C) ** -0.5

_CACHE = {}


def _build(reps=1):
    nc = bacc.Bacc(None, target_bir_lowering=False)

    def inp(name, shape, dt):
        return nc.declare_dram_parameter(name, list(shape), dt, isOutput=False)

    xl = inp("xl", (TL, C), FP32)          # own tokens, token-major
    xlT = inp("xlT", (C, TL), F32R)        # own tokens, feature-major
    wq = inp("wq", (C, C), F32R)           # [C, H*HD]
    wk = inp("wk", (C, C), F32R)
    wv = inp("wv", (C, C), F32R)
    wo = inp("wo", (C, C), F32R)
    bo = inp("bo", (1, C), F32R)
    masks = inp("masks", (NCORES, P, P), FP32)   # per-source-rank diag masks
    wrwn = inp("wrwn", (C, 2 * E), FP32)
    brbn = inp("brbn", (1, 2 * E), FP32)
    noise = inp("noise", (TL, E), FP32)
    esel = inp("esel", (1, E), FP32)             # one-hot expert selector
    ln1g = inp("ln1g", (1, C), FP32)
    ln1b = inp("ln1b", (1, C), FP32)
    ln2g = inp("ln2g", (1, C), FP32)
    ln2b = inp("ln2b", (1, C), FP32)
    w1 = inp("w1", (C, FF), BF16)                # own expert
    b1 = inp("b1", (FF,), FP32)
    w2 = inp("w2", (FF, C), BF16)
    b2 = inp("b2", (1, C), BF16)

    out_sl = nc.declare_dram_parameter("out_slice", [TL, C], FP32, isOutput=True)

    # internal DRAM for collectives
    cc_kT_in = nc.dram_tensor("cc_kT_in", [C, TL], FP16)
    cc_kT_out = nc.dram_tensor("cc_kT_out", [NCORES * C, TL], FP16,
                               addr_space="Shared")
    cc_v_in = nc.dram_tensor("cc_v_in", [TL, VA], FP16)
    cc_v_out = nc.dram_tensor("cc_v_out", [NCORES * TL, VA], FP16,
                              addr_space="Shared")
    cc_g_in = nc.dram_tensor("cc_g_in", [TL, E], FP32)
    cc_g_out = nc.dram_tensor("cc_g_out", [NCORES * TL, E], FP32,
                              addr_space="Shared")
    cc_x_in = nc.dram_tensor("cc_x_in", [C, TL], BF16)
    cc_x_out = nc.dram_tensor("cc_x_out", [NCORES * C, TL], BF16,
                              addr_space="Shared")
    cc_rs_in = nc.dram_tensor("cc_rs_in", [NCORES * TL, C], BF16)
    cc_rs_out = nc.dram_tensor("cc_rs_out", [TL, C], BF16)
    RG = [list(range(NCORES))]

    def bcast(handle, rows, cols):
        a = handle.ap()
        return bass.AP(tensor=a.tensor, offset=a.offset,
                       ap=[[0, rows]] + list(a.ap)[1:])

    with tile.TileContext(nc) as tc, ExitStack() as octx:
        const = octx.enter_context(tc.tile_pool(name="const", bufs=1))
        ident = const.tile([P, P], FP32)
        make_identity(nc, ident)
        ones_f = const.tile([1, P], FP32)
        nc.vector.memset(ones_f, 1.0)
        ones_r = const.tile([1, P], F32R)
        nc.vector.tensor_copy(out=ones_r[:], in_=ones_f[:])
        ones2 = const.tile([P, H, 2], FP16)
        nc.vector.memset(ones2, 1.0)
        ones_h = const.tile([1, P], BF16)
        nc.vector.memset(ones_h, 1.0)
        eps_t = const.tile([P, 1], FP32)
        nc.vector.memset(eps_t, 1e-5)
        g1b = const.tile([P, C], FP32)
        nc.gpsimd.dma_start(out=g1b, in_=bcast(ln1g, P, C))
        b1b = const.tile([P, C], FP32)
        nc.gpsimd.dma_start(out=b1b, in_=bcast(ln1b, P, C))
        g2b = const.tile([P, C], FP32)
        nc.gpsimd.dma_start(out=g2b, in_=bcast(ln2g, P, C))
        b2b = const.tile([P, C], FP32)
        nc.gpsimd.dma_start(out=b2b, in_=bcast(ln2b, P, C))
        brbn_b = const.tile([P, 2 * E], FP32)
        nc.gpsimd.dma_start(out=brbn_b, in_=bcast(brbn, P, 2 * E))
        esel_b = const.tile([P, E], FP32)
        nc.gpsimd.dma_start(out=esel_b, in_=bcast(esel, P, E))
        mask_t = const.tile([P, NCORES, P], FP32)
        nc.sync.dma_start(out=mask_t,
                          in_=masks.ap().rearrange("r p q -> p r q"))
        bo_sb = const.tile([1, C], F32R)
        nc.sync.dma_start(out=bo_sb, in_=bo.ap())
        b2_sb = const.tile([1, C], BF16)
        nc.sync.dma_start(out=b2_sb, in_=b2.ap())
        wrwn_sb = const.tile([P, CF, 2 * E], FP32)
        nc.sync.dma_start(out=wrwn_sb,
                          in_=wrwn.ap().rearrange("(f p) e -> p f e", p=P))
        b1_sb = const.tile([P, FF // P], FP32)
        nc.sync.dma_start(out=b1_sb,
                          in_=b1.ap().rearrange("(o p) -> p o", p=P))

        persist = octx.enter_context(tc.tile_pool(name="persist", bufs=1))
        qT = [persist.tile([P, TL], FP16, name=f"qT{m}") for m in range(CF)]
        x1sb = [persist.tile([P, C], FP32, name=f"x1_{m}") for m in range(4)]
        attn_sb = [persist.tile([P, C], FP32, name=f"attn{m}") for m in range(4)]

        for _rep in range(reps):
         # ---------------- Phase A: q/k/v projections ----------------
         with ExitStack() as ctx:
             pa = ctx.enter_context(tc.tile_pool(name="pa", bufs=1))
             ev = ctx.enter_context(tc.tile_pool(name="ev", bufs=4))
             pps = ctx.enter_context(tc.tile_pool(name="pps", bufs=4, space="PSUM"))
             xt = []
             for f in range(CF):
                 t = pa.tile([P, TL], F32R, name=f"xt{f}")
                 nc.sync.dma_start(out=t, in_=xlT.ap()[f * P:(f + 1) * P, :])
                 xt.append(t)
             with ExitStack() as c2:
                 pw = c2.enter_context(tc.tile_pool(name="pwv", bufs=1))
                 wv_t = []
                 for f in range(CF):
                     t = pw.tile([P, C], F32R, name=f"wv{f}")
                     nc.sync.dma_start(out=t, in_=wv.ap()[f * P:(f + 1) * P, :])
                     wv_t.append(t)
                 for mt in range(4):
                     va = ev.tile([P, H, HD + 2], FP16, name="vaug")
                     nc.vector.tensor_copy(out=va[:, :, HD:HD + 2], in_=ones2[:])
                     for n in range(2):
                         ps_ = pps.tile([P, 512], FP32, name="v_ps")
                         for f in range(CF):
                             nc.tensor.matmul(
                                 ps_[:], xt[f][:, mt * P:(mt + 1) * P],
                                 wv_t[f][:, n * 512:(n + 1) * 512],
                                 start=(f == 0), stop=(f == CF - 1))
                         nc.vector.tensor_copy(
                             out=va[:, n * 8:(n + 1) * 8, 0:HD],
                             in_=ps_[:].rearrange("p (h d) -> p h d", d=HD))
                     nc.sync.dma_start(out=cc_v_in.ap()[mt * P:(mt + 1) * P, :],
                                       in_=va[:].rearrange("p h d -> p (h d)"))
             nc.gpsimd.collective_compute(
                 "AllGather", OP.bypass, replica_groups=RG,
                 ins=[cc_v_in.ap()], outs=[cc_v_out.ap()])
             with ExitStack() as c2:
                 pw = c2.enter_context(tc.tile_pool(name="pwk", bufs=1))
                 wk_t = []
                 for f in range(CF):
                     t = pw.tile([P, C], F32R, name=f"wk{f}")
                     nc.sync.dma_start(out=t, in_=wk.ap()[f * P:(f + 1) * P, :])
                     wk_t.append(t)
                 for m in range(CF):
                     ps2 = pps.tile([P, TL], FP32, name="qk_ps")
                     for f in range(CF):
                         nc.tensor.matmul(ps2[:], wk_t[f][:, m * P:(m + 1) * P],
                                          xt[f][:], start=(f == 0),
                                          stop=(f == CF - 1))
                     kt_ev = ev.tile([P, TL], FP16, name="kt_ev")
                     nc.vector.tensor_copy(out=kt_ev[:], in_=ps2[:])
                     nc.sync.dma_start(out=cc_kT_in.ap()[m * P:(m + 1) * P, :],
                                       in_=kt_ev[:])
             nc.gpsimd.collective_compute(
                 "AllGather", OP.bypass, replica_groups=RG,
                 ins=[cc_kT_in.ap()], outs=[cc_kT_out.ap()])
             with ExitStack() as c2:
                 pw = c2.enter_context(tc.tile_pool(name="pwq", bufs=1))
                 wq_t = []
                 for f in range(CF):
                     t = pw.tile([P, C], F32R, name=f"wq{f}")
                     nc.sync.dma_start(out=t, in_=wq.ap()[f * P:(f + 1) * P, :])
                     wq_t.append(t)
                 for m in range(CF):
                     ps_ = pps.tile([P, TL], FP32, name="qk_ps")
                     for f in range(CF):
                         nc.tensor.matmul(ps_[:], wq_t[f][:, m * P:(m + 1) * P],
                                          xt[f][:], start=(f == 0),
                                          stop=(f == CF - 1))
                     nc.vector.tensor_copy(out=qT[m][:], in_=ps_[:])

         # ---------------- Phase B1: causal attention ----------------
         with ExitStack() as ctx:
             ktp = ctx.enter_context(tc.tile_pool(name="ktp", bufs=10))
             vtp = ctx.enter_context(tc.tile_pool(name="vtp", bufs=24))
             ex0p = ctx.enter_context(tc.tile_pool(name="ex0p", bufs=18))
             ex1p = ctx.enter_context(tc.tile_pool(name="ex1p", bufs=18))
             tmpp = ctx.enter_context(tc.tile_pool(name="tmpp", bufs=6))
             sps = ctx.enter_context(tc.tile_pool(name="sps", bufs=2, space="PSUM"))
             s1s = ctx.enter_context(tc.tile_pool(name="s1s", bufs=2, space="PSUM"))
             avs = ctx.enter_context(tc.tile_pool(name="avs", bufs=3, space="PSUM"))
             for b in range(B):
                 for hp in range(H // 2):
                     kts, vts = {}, {}
                     for r in range(NCORES):
                         kt = ktp.tile([P, TB], FP16, name="kt")
                         nc.sync.dma_start(
                             out=kt,
                             in_=cc_kT_out.ap()[r * C + hp * P:r * C + (hp + 1) * P,
                                                b * TB:(b + 1) * TB])
                         kts[r] = kt
                         for ik in range(2):
                             vt = vtp.tile([P, 132], FP16, name="vt")
                             row0 = r * TL + b * TB + ik * P
                             nc.sync.dma_start(
                                 out=vt,
                                 in_=cc_v_out.ap()[row0:row0 + P,
                                                   hp * 132:(hp + 1) * 132])
                             vts[(r, ik)] = vt
                     av0 = avs.tile([P, 132], FP32, name="av")
                     av1 = avs.tile([P, 132], FP32, name="av")
                     exps0, exps1 = {}, {}
                     for r in range(NCORES):
                         for hh in range(2):
                             bp = hh * HD
                             sp = sps.tile([P, TB], FP32, name="sc")
                             nc.tensor.matmul(
                                 sp[:], kts[r][bp:bp + HD, 0:P],
                                 qT[hp][bp:bp + HD, b * TB:(b + 1) * TB],
                                 start=True, stop=True)
                             e0 = ex0p.tile([P, TB], FP16, name="e0")
                             nc.scalar.activation(e0[:, P:TB], sp[:, P:TB],
                                                  ACT.Exp, scale=SCALE)
                             tmp = tmpp.tile([P, P], FP32, name="tmp")
                             nc.scalar.activation(tmp[:], sp[:, 0:P],
                                                  ACT.Exp, scale=SCALE)
                             nc.vector.tensor_mul(e0[:, 0:P], tmp[:],
                                                  mask_t[:, r, :])
                             exps0[(r, hh)] = e0
                             s1 = s1s.tile([P, P], FP32, name="s1")
                             nc.tensor.matmul(
                                 s1[:], kts[r][bp:bp + HD, P:TB],
                                 qT[hp][bp:bp + HD, b * TB + P:(b + 1) * TB],
                                 start=True, stop=True)
                             tmp2 = tmpp.tile([P, P], FP32, name="tmp")
                             nc.scalar.activation(tmp2[:], s1[:],
                                                  ACT.Exp, scale=SCALE)
                             e1 = ex1p.tile([P, P], FP16, name="e1")
                             nc.vector.tensor_mul(e1[:], tmp2[:], mask_t[:, r, :])
                             exps1[(r, hh)] = e1
                     for hh in range(2):
                         co = hh * (HD + 2)
                         for r in range(NCORES):
                             nc.tensor.matmul(
                                 av0[:, co:co + HD + 2], exps0[(r, hh)][:, 0:P],
                                 vts[(r, 0)][:, co:co + HD + 2],
                                 start=(r == 0), stop=(r == NCORES - 1))
                         for r in range(NCORES):
                             nc.tensor.matmul(
                                 av1[:, co:co + HD + 2], exps0[(r, hh)][:, P:TB],
                                 vts[(r, 0)][:, co:co + HD + 2],
                                 start=(r == 0), stop=False)
                             nc.tensor.matmul(
                                 av1[:, co:co + HD + 2], exps1[(r, hh)][:],
                                 vts[(r, 1)][:, co:co + HD + 2],
                                 start=False, stop=(r == NCORES - 1))
                     for q, avp in enumerate((av0, av1)):
                         mtt = b * 2 + q
                         rc = tmpp.tile([P, 2], FP32, name="rc")
                         den = avp[:].rearrange("p (h s) -> p h s", s=HD + 2)[:, :, HD]
                         nc.vector.reciprocal(rc[:], den)
                         for hh in range(2):
                             h = 2 * hp + hh
                             nc.vector.tensor_scalar_mul(
                                 attn_sb[mtt][:, h * HD:(h + 1) * HD],
                                 avp[:, hh * (HD + 2):hh * (HD + 2) + HD],
                                 rc[:, hh:hh + 1])

         # ---------------- Phase B2: proj + LN1 + router ----------------
         with ExitStack() as ctx:
             pw = ctx.enter_context(tc.tile_pool(name="pb2", bufs=1))
             evp = ctx.enter_context(tc.tile_pool(name="evb2", bufs=4))
             tps = ctx.enter_context(tc.tile_pool(name="tps", bufs=3, space="PSUM"))
             prs = ctx.enter_context(tc.tile_pool(name="prs", bufs=2, space="PSUM"))
             wo_t = []
             for f in range(CF):
                 t = pw.tile([P, C], F32R, name=f"wo{f}")
                 nc.sync.dma_start(out=t, in_=wo.ap()[f * P:(f + 1) * P, :])
                 wo_t.append(t)
             attnT = [pw.tile([P, TL], F32R, name=f"attnT{f}") for f in range(CF)]
             for mt in range(4):
                 for f in range(CF):
                     tp = tps.tile([P, P], FP32, name="tp")
                     nc.tensor.transpose(tp[:], attn_sb[mt][:, f * P:(f + 1) * P],
                                         ident[:])
                     nc.vector.tensor_copy(out=attnT[f][:, mt * P:(mt + 1) * P],
                                           in_=tp[:])
             x1T_f = [pw.tile([P, TL], FP32, name=f"x1Tf{f}") for f in range(CF)]
             x1T_h = [pw.tile([P, TL], BF16, name=f"x1Th{f}") for f in range(CF)]
             for mt in range(4):
                 p_sb = evp.tile([P, C], FP32, name="p_sb")
                 for n in range(2):
                     pp = prs.tile([P, 512], FP32, name="pp")
                     nc.tensor.matmul(pp[:], ones_r[:],
                                      bo_sb[:, n * 512:(n + 1) * 512],
                                      start=True, stop=False)
                     for f in range(CF):
                         nc.tensor.matmul(pp[:], attnT[f][:, mt * P:(mt + 1) * P],
                                          wo_t[f][:, n * 512:(n + 1) * 512],
                                          start=False, stop=(f == CF - 1))
                     nc.scalar.copy(out=p_sb[:, n * 512:(n + 1) * 512], in_=pp[:])
                 # LN1 + residual
                 stats = evp.tile([P, 2, 6], FP32, name="stats")
                 for sg in range(2):
                     nc.vector.bn_stats(out=stats[:, sg, :],
                                        in_=p_sb[:, sg * 512:(sg + 1) * 512])
                 mv = evp.tile([P, 2], FP32, name="mv")
                 nc.vector.bn_aggr(out=mv[:], in_=stats[:])
                 std = evp.tile([P, 1], FP32, name="std")
                 nc.scalar.activation(out=std[:], in_=mv[:, 1:2], func=ACT.Sqrt,
                                      bias=eps_t[:])
                 rstd = evp.tile([P, 1], FP32, name="rstd")
                 nc.vector.reciprocal(rstd[:], std[:])
                 xt_l = evp.tile([P, C], FP32, name="xt_l")
                 nc.sync.dma_start(out=xt_l, in_=xl.ap()[mt * P:(mt + 1) * P, :])
                 t1 = evp.tile([P, C], FP32, name="t1")
                 nc.vector.tensor_scalar(out=t1[:], in0=p_sb[:],
                                         scalar1=mv[:, 0:1], scalar2=rstd[:],
                                         op0=OP.subtract, op1=OP.mult)
                 nc.vector.tensor_mul(t1[:], t1[:], g1b[:])
                 nc.vector.tensor_add(t1[:], t1[:], b1b[:])
                 nc.vector.tensor_add(x1sb[mt][:], t1[:], xt_l[:])
                 # x1 transposes: fp32 (router) + bf16 (expert AG)
                 for f in range(CF):
                     tp = tps.tile([P, P], FP32, name="tp")
                     nc.tensor.transpose(tp[:], x1sb[mt][:, f * P:(f + 1) * P],
                                         ident[:])
                     nc.vector.tensor_copy(out=x1T_f[f][:, mt * P:(mt + 1) * P],
                                           in_=tp[:])
                     nc.scalar.copy(out=x1T_h[f][:, mt * P:(mt + 1) * P],
                                    in_=tp[:])
             for f in range(CF):
                 nc.sync.dma_start(out=cc_x_in.ap()[f * P:(f + 1) * P, :],
                                   in_=x1T_h[f][:])
             # router (exact fp32)
             for mt in range(4):
                 rp = prs.tile([P, 2 * E], FP32, name="rp")
                 for f in range(CF):
                     nc.tensor.matmul(rp[:], x1T_f[f][:, mt * P:(mt + 1) * P],
                                      wrwn_sb[:, f, :], start=(f == 0),
                                      stop=(f == CF - 1))
                 lg = evp.tile([P, 2 * E], FP32, name="lg")
                 nc.vector.tensor_add(lg[:], rp[:], brbn_b[:])
                 e_ = evp.tile([P, E], FP32, name="e_")
                 nc.scalar.activation(out=e_[:], in_=lg[:, E:2 * E],
                                      func=ACT.Exp)
                 sp_ = evp.tile([P, E], FP32, name="sp_")
                 nc.scalar.activation(out=sp_[:], in_=e_[:], func=ACT.Ln,
                                      bias=1.0)
                 nz = evp.tile([P, E], FP32, name="nz")
                 nc.sync.dma_start(out=nz, in_=noise.ap()[mt * P:(mt + 1) * P, :])
                 noisy = evp.tile([P, E], FP32, name="noisy")
                 nc.vector.tensor_mul(noisy[:], nz[:], sp_[:])
                 nc.vector.tensor_add(noisy[:], noisy[:], lg[:, 0:E])
                 m1 = evp.tile([P, 1], FP32, name="m1")
                 nc.vector.reduce_max(m1[:], noisy[:], axis=AX.X)
                 nm1 = evp.tile([P, 1], FP32, name="nm1")
                 nc.vector.tensor_scalar_mul(nm1[:], m1[:], -1.0)
                 eq = evp.tile([P, E], FP32, name="eq")
                 nc.vector.tensor_scalar(out=eq[:], in0=noisy[:], scalar1=m1[:],
                                         scalar2=None, op0=OP.is_equal)
                 t2 = evp.tile([P, E], FP32, name="t2")
                 nc.vector.tensor_scalar_mul(t2[:], eq[:], -1e30)
                 nc.vector.tensor_add(t2[:], t2[:], noisy[:])
                 m2 = evp.tile([P, 1], FP32, name="m2")
                 nc.vector.reduce_max(m2[:], t2[:], axis=AX.X)
                 d = evp.tile([P, E], FP32, name="d")
                 nc.scalar.activation(out=d[:], in_=noisy[:], func=ACT.Exp,
                                      bias=nm1[:])
                 em2 = evp.tile([P, 1], FP32, name="em2")
                 nc.scalar.activation(out=em2[:], in_=m2[:], func=ACT.Exp,
                                      bias=nm1[:])
                 den_ = evp.tile([P, 1], FP32, name="den_")
                 nc.scalar.add(out=den_[:], in_=em2[:], add=1.0)
                 rden = evp.tile([P, 1], FP32, name="rden")
                 nc.vector.reciprocal(rden[:], den_[:])
                 ge_ = evp.tile([P, E], FP32, name="ge_")
                 nc.vector.tensor_scalar(out=ge_[:], in0=noisy[:], scalar1=m2[:],
                                         scalar2=None, op0=OP.is_ge)
                 gt = evp.tile([P, E], FP32, name="gt")
                 nc.vector.tensor_mul(gt[:], d[:], ge_[:])
                 nc.vector.tensor_scalar_mul(gt[:], gt[:], rden[:])
                 nc.sync.dma_start(out=cc_g_in.ap()[mt * P:(mt + 1) * P, :],
                                   in_=gt[:])
             nc.gpsimd.collective_compute(
                 "AllGather", OP.bypass, replica_groups=RG,
                 ins=[cc_g_in.ap()], outs=[cc_g_out.ap()])
             nc.gpsimd.collective_compute(
                 "AllGather", OP.bypass, replica_groups=RG,
                 ins=[cc_x_in.ap()], outs=[cc_x_out.ap()])

         # ---------------- Phase C: expert FFN (dense) + RS + LN2 ----------------
         with ExitStack() as ctx:
             xcp = ctx.enter_context(tc.tile_pool(name="xcp", bufs=12))
             htp = ctx.enter_context(tc.tile_pool(name="htp", bufs=20))
             gp = ctx.enter_context(tc.tile_pool(name="gp", bufs=6))
             ctp = ctx.enter_context(tc.tile_pool(name="ctp", bufs=4))
             hs = ctx.enter_context(tc.tile_pool(name="hs", bufs=2, space="PSUM"))
             es = ctx.enter_context(tc.tile_pool(name="es", bufs=4, space="PSUM"))
             for psi in range(2):
                 with ExitStack() as c2:
                     pw = c2.enter_context(tc.tile_pool(name=f"pwf{psi}", bufs=1))
                     w1h = []
                     for f in range(CF):
                         t = pw.tile([P, FFH], BF16, name=f"w1h{f}")
                         nc.sync.dma_start(
                             out=t, in_=w1.ap()[f * P:(f + 1) * P,
                                                psi * FFH:(psi + 1) * FFH])
                         w1h.append(t)
                     w2h = []
                     for k in range(FFH // P):
                         t = pw.tile([P, C], BF16, name=f"w2h{k}")
                         r0 = psi * FFH + k * P
                         nc.sync.dma_start(out=t, in_=w2.ap()[r0:r0 + P, :])
                         w2h.append(t)
                     for cch in range(NCORES):
                         x1c = []
                         for f in range(CF):
                             t = xcp.tile([P, TL], BF16, name="x1c")
                             nc.sync.dma_start(
                                 out=t,
                                 in_=cc_x_out.ap()[cch * C + f * P:
                                                   cch * C + (f + 1) * P, :])
                             x1c.append(t)
                         hts = []
                         for k in range(FFH // P):
                             hps = hs.tile([P, TL], FP32, name="hps")
                             for f in range(CF):
                                 nc.tensor.matmul(hps[:],
                                                  w1h[f][:, k * P:(k + 1) * P],
                                                  x1c[f][:], start=(f == 0),
                                                  stop=(f == CF - 1))
                             ht = htp.tile([P, TL], BF16, name="ht")
                             kk = psi * (FFH // P) + k
                             nc.scalar.activation(out=ht[:], in_=hps[:],
                                                  func=ACT.Gelu,
                                                  bias=b1_sb[:, kk:kk + 1])
                             hts.append(ht)
                         for m in range(4):
                             gt_ = gp.tile([P, E], FP32, name="gt_")
                             nc.sync.dma_start(
                                 out=gt_,
                                 in_=cc_g_out.ap()[cch * TL + m * P:
                                                   cch * TL + (m + 1) * P, :])
                             tg = gp.tile([P, E], FP32, name="tg")
                             nc.vector.tensor_mul(tg[:], gt_[:], esel_b[:])
                             gcol = gp.tile([P, 1], FP32, name="gcol")
                             nc.vector.reduce_sum(gcol[:], tg[:], axis=AX.X)
                             # both n-halves accumulate together so each
                             # hts[k] lhsT is reused by two consecutive MMs
                             eops = [es.tile([P, 512], FP32, name="eop")
                                     for _ in range(2)]
                             for n in range(2):
                                 nc.tensor.matmul(eops[n][:], ones_h[:],
                                                  b2_sb[:, n * 512:(n + 1) * 512],
                                                  start=True, stop=False)
                             for k in range(FFH // P):
                                 for n in range(2):
                                     nc.tensor.matmul(
                                         eops[n][:], hts[k][:, m * P:(m + 1) * P],
                                         w2h[k][:, n * 512:(n + 1) * 512],
                                         start=False, stop=(k == FFH // P - 1))
                             for n in range(2):
                                 cb = ctp.tile([P, 512], BF16, name="cb")
                                 nc.vector.tensor_scalar_mul(cb[:], eops[n][:],
                                                             gcol[:])
                                 dst = cc_rs_in.ap()[cch * TL + m * P:
                                                     cch * TL + (m + 1) * P,
                                                     n * 512:(n + 1) * 512]
                                 if psi == 0:
                                     nc.sync.dma_start(out=dst, in_=cb[:])
                                 else:
                                     nc.gpsimd.dma_start(out=dst, in_=cb[:],
                                                         accum_op=OP.add)
             nc.gpsimd.collective_compute(
                 "ReduceScatter", OP.add, replica_groups=RG,
                 ins=[cc_rs_in.ap()], outs=[cc_rs_out.ap()])
             # LN2 + residual + output
             mp = ctx.enter_context(tc.tile_pool(name="mp", bufs=4))
             for mt in range(4):
                 mo_h = mp.tile([P, C], BF16, name="mo_h")
                 nc.sync.dma_start(out=mo_h,
                                   in_=cc_rs_out.ap()[mt * P:(mt + 1) * P, :])
                 mo = mp.tile([P, C], FP32, name="mo")
                 nc.vector.tensor_copy(out=mo[:], in_=mo_h[:])
                 stats = mp.tile([P, 2, 6], FP32, name="stats2")
                 for sg in range(2):
                     nc.vector.bn_stats(out=stats[:, sg, :],
                                        in_=mo[:, sg * 512:(sg + 1) * 512])
                 mv = mp.tile([P, 2], FP32, name="mv2")
                 nc.vector.bn_aggr(out=mv[:], in_=stats[:])
                 std = mp.tile([P, 1], FP32, name="std2")
                 nc.scalar.activation(out=std[:], in_=mv[:, 1:2], func=ACT.Sqrt,
                                      bias=eps_t[:])
                 rstd = mp.tile([P, 1], FP32, name="rstd2")
                 nc.vector.reciprocal(rstd[:], std[:])
                 t1 = mp.tile([P, C], FP32, name="t1o")
                 nc.vector.tensor_scalar(out=t1[:], in0=mo[:],
                                         scalar1=mv[:, 0:1], scalar2=rstd[:],
                                         op0=OP.subtract, op1=OP.mult)
                 nc.vector.tensor_mul(t1[:], t1[:], g2b[:])
                 nc.vector.tensor_add(t1[:], t1[:], b2b[:])
                 nc.vector.tensor_add(t1[:], t1[:], x1sb[mt][:])
                 nc.sync.dma_start(out=out_sl.ap()[mt * P:(mt + 1) * P, :],
                                   in_=t1[:])

    nc.compile()
    return nc


def _make_in_maps(x, Wq, Wk, Wv, Wo, bo, ln1_g, ln1_b, Wr, br, Wn, bn,
                  W1, b1, W2, b2, ln2_g, ln2_b, noise):
    x = np.asarray(x, np.float32)
    noise = np.asarray(noise, np.float32)
    wq_f = np.ascontiguousarray(
        np.transpose(np.asarray(Wq, np.float32), (1, 0, 2)).reshape(C, C))
    wk_f = np.ascontiguousarray(
        np.transpose(np.asarray(Wk, np.float32), (1, 0, 2)).reshape(C, C))
    wv_f = np.ascontiguousarray(
        np.transpose(np.asarray(Wv, np.float32), (1, 0, 2)).reshape(C, C))
    wo_f = np.ascontiguousarray(np.asarray(Wo, np.float32))
    wrwn_f = np.ascontiguousarray(
        np.concatenate([np.asarray(Wr, np.float32),
                        np.asarray(Wn, np.float32)], axis=1))
    brbn_f = np.concatenate([np.asarray(br, np.float32),
                             np.asarray(bn, np.float32)]).reshape(1, 2 * E)
    W1 = np.asarray(W1, np.float32)
    W2 = np.asarray(W2, np.float32)
    b1 = np.asarray(b1, np.float32)
    b2 = np.asarray(b2, np.float32)

    in_maps = []
    for c in range(NCORES):
        xlc = np.ascontiguousarray(
            np.concatenate([x[0, c::NCORES, :], x[1, c::NCORES, :]], axis=0))
        nzc = np.ascontiguousarray(
            np.concatenate([noise[0, c::NCORES, :], noise[1, c::NCORES, :]],
                           axis=0))
        mk = np.empty((NCORES, P, P), np.float32)
        for r in range(NCORES):
            mk[r] = np.triu(np.ones((P, P), np.float32),
                            0 if r <= c else 1)
        es_ = np.zeros((1, E), np.float32)
        es_[0, c] = 1.0
        in_maps.append({
            "xl": xlc,
            "xlT": np.ascontiguousarray(xlc.T),
            "wq": wq_f, "wk": wk_f, "wv": wv_f, "wo": wo_f,
            "bo": np.asarray(bo, np.float32).reshape(1, C),
            "masks": mk,
            "wrwn": wrwn_f, "brbn": brbn_f,
            "noise": nzc, "esel": es_,
            "ln1g": np.asarray(ln1_g, np.float32).reshape(1, C),
            "ln1b": np.asarray(ln1_b, np.float32).reshape(1, C),
            "ln2g": np.asarray(ln2_g, np.float32).reshape(1, C),
            "ln2b": np.asarray(ln2_b, np.float32).reshape(1, C),
            "w1": W1[c].astype(ml_dtypes.bfloat16),
            "b1": b1[c],
            "w2": W2[c].astype(ml_dtypes.bfloat16),
            "b2": b2[c].reshape(1, C).astype(ml_dtypes.bfloat16),
        })

    return in_maps


def kernel(**inputs):
    if "nc" not in _CACHE:
        _CACHE["nc"] = _build()
    nc = _CACHE["nc"]
    in_maps = _make_in_maps(**inputs)
    res = run_bass_kernel_spmd(nc, in_maps, list(range(NCORES)))
    out = np.empty((B, T, C), np.float32)
    for c in range(NCORES):
        sl = np.asarray(res.results[c]["out_slice"], np.float32)
        out[0, c::NCORES, :] = sl[:TB]
        out[1, c::NCORES, :] = sl[TB:]
    return out

